# revision 24
# baseline (speedup 1.0000x reference)
"""TRN2 Bass kernel for nn_NeuralNetworkModel_tanh_aniso_wm.

Model (per point):
    H1 = tanh(x @ W1)           x:[N,9]  W1:[9,256]
    H2 = tanh(H1 @ W3)          W3:[256,256]
    H3 = tanh(y @ W2)           y:[N,6]  W2:[6,256]
    pp = H2[:, :128] * H3[:, :128];  qq = H2[:, 128:] * H3[:, 128:]
    op = pp @ Wp (5 cols: f1,f2,f3,wm1,wm2); oq = qq @ Wq (4: f4,f5,wm3,wm4)
    out = [elu(f1,f1), elu(f2,f2), elu(f2,f3), elu(f2,f4), elu(f2,f5),
           wm1, wm2, wm3, wm4]
    with elu(pos,g) = relu(pos) + min(exp(g)-1, 0)

Design: pure data parallel over 8 cores (32768 points each). Activations
are kept feature-major ([feature, point]) so the tiny head matmuls stream
points with weights stationary. All matmuls run in float32r (1 cycle/row
at moving>=256, ~1.5e-4 rel precision). tanh/exp run on the ACT engine
reading PSUM directly; elementwise work on the vector engine. Chunks of
C=512 points flow through a software-pipelined loop: stage A (input DMA,
layer-1 matmuls, tanh) for chunk i+1 is emitted before stage B (layer-2,
products, heads, ELU, output DMA) of chunk i so the PE starts the next
chunk's matmuls as soon as the previous tanh drains.

Constraints baked in: compute-engine APs start only at partitions
0/32/64/96; matmul outputs only at partition 0; at most ONE semaphore
wait per instruction (split into NOP chains post-scheduling).
"""
import contextlib

import numpy as np

import concourse.bass as bass
import concourse.mybir as mybir
import concourse.tile as tile
from concourse import bass_utils

AF = mybir.ActivationFunctionType
ALU = mybir.AluOpType
F32 = mybir.dt.float32
F32R = mybir.dt.float32r

N_TOTAL = 262144
N_CORES = 8
NPC = N_TOTAL // N_CORES  # 32768 points per core
C = 512                   # points per chunk
NCH = NPC // C            # 64 chunks

DEFAULT_CFG = dict(
    c=512,             # points per chunk
    pq_pool=False,     # run the pp|qq multiply on GpSimd instead of DVE
    pq_split=False,    # pp on DVE, qq on GpSimd
    head_group=2,      # chunks sharing one head psum tile / exp / out DMA
    work_bufs=3,       # h13/h2/pq buffer depth
    small_bufs=3,      # e/t/out9 buffer depth
    zin_bufs=4,        # input-tile prefetch depth
    pipelined=True,    # emit stage A of chunk i+1 before stage B of chunk i
    ahead=1,           # pipeline depth of stage A emission
    split_pa=False,    # layer-1 psum as 2x[...] (bufs=2) vs 1x[...] (bufs=1)
    pa_bufs=1, p2_bufs=1, ph_bufs=1,
)


# ---------------------------------------------------------------------------
# Wait-splitting pass: this toolchain encodes at most ONE semaphore wait per
# instruction (walrus raises "Too many sync wait commands" otherwise), but
# Tile's wait assignment can attach several. Engines execute their stream in
# order, so moving all but one wait onto same-engine NOPs directly before the
# instruction preserves semantics.
_WSPLIT_CTR = [0]


def _split_multi_waits(nc):
    n_split = 0
    for f in nc.m.functions:
        for blk in f.blocks:
            out = []
            changed = False
            for inst in blk.instructions:
                si = inst.sync_info
                if si is not None and si.on_wait is not None and len(si.on_wait) > 1:
                    waits = list(si.on_wait)
                    for w in waits[:-1]:
                        _WSPLIT_CTR[0] += 1
                        nop = mybir.InstNoOp(name=f"I-wsplit-{_WSPLIT_CTR[0]}")
                        nop.engine = inst.engine
                        nop.sync_info = mybir.SyncInfo(on_wait=[w], on_update=[])
                        nc.register_instruction(nop)
                        out.append(nop)
                    inst.sync_info = mybir.SyncInfo(
                        on_wait=[waits[-1]], on_update=list(si.on_update or [])
                    )
                    changed = True
                    n_split += 1
                out.append(inst)
            if changed:
                blk.instructions = out
    return n_split


class _Stages:
    """Per-chunk pipeline stages; state passed from A to B via dicts."""

    def __init__(self, nc, pools, w, dram, cfg):
        self.nc = nc
        self.pools = pools
        self.w = w
        self.dram = dram
        self.cfg = cfg
        self.C = cfg["c"]

    def stage_a(self, ch):
        """Input DMA, layer-1 (+3) matmuls, tanh -> h13."""
        nc, w, cfg = self.nc, self.w, self.cfg
        C = self.C
        zin, work, small, psum = self.pools
        s = ch * C
        zt = zin.tile([15, C], F32R, tag="zt")
        nc.sync.dma_start(zt[:], self.dram["zt"][:, s:s + C])

        h13 = work.tile([128, 4 * C], F32R, tag="h13")
        if cfg["split_pa"]:
            for half in range(2):
                pah = psum.tile([128, 2 * C], F32, tag="pa", bufs=2)
                for m in range(2):
                    mm = 2 * half + m
                    nc.tensor.matmul(
                        pah[:, m * C:(m + 1) * C],
                        w["wz"][:, mm * 128:(mm + 1) * 128],
                        zt[:], start=True, stop=True)
                nc.scalar.activation(h13[:, half * 2 * C:(half + 1) * 2 * C],
                                     pah[:], AF.Tanh)
        else:
            pa = psum.tile([128, 4 * C], F32, tag="pa", bufs=cfg["pa_bufs"])
            for m in range(4):
                nc.tensor.matmul(
                    pa[:, m * C:(m + 1) * C],
                    w["wz"][:, m * 128:(m + 1) * 128],
                    zt[:], start=True, stop=True)
            nc.scalar.activation(h13[:], pa[:], AF.Tanh)
        return {"h13": h13}

    def stage_b1(self, ch, st):
        """Layer-2 matmuls, tanh, pp|qq products."""
        nc, w, cfg = self.nc, self.w, self.cfg
        C = self.C
        zin, work, small, psum = self.pools
        h13 = st["h13"]

        p2 = psum.tile([128, 2 * C], F32, tag="p2", bufs=cfg["p2_bufs"])
        for m in range(2):
            nc.tensor.matmul(p2[:, m * C:(m + 1) * C],
                             w["w3a"][:, m * 128:(m + 1) * 128],
                             h13[:, 0:C], start=True, stop=False)
            nc.tensor.matmul(p2[:, m * C:(m + 1) * C],
                             w["w3b"][:, m * 128:(m + 1) * 128],
                             h13[:, C:2 * C], start=False, stop=True)
        h2 = work.tile([128, 2 * C], F32R, tag="h2")
        nc.scalar.activation(h2[:], p2[:], AF.Tanh)

        # pp | qq product
        pq = work.tile([128, 2 * C], F32R, tag="pq")
        if cfg["pq_split"]:
            # pp on DVE, qq on the otherwise-idle GpSimd
            nc.vector.tensor_mul(pq[:, 0:C], h2[:, 0:C], h13[:, 2 * C:3 * C])
            nc.gpsimd.tensor_mul(pq[:, C:2 * C], h2[:, C:2 * C],
                                 h13[:, 3 * C:4 * C])
        else:
            eng = nc.gpsimd if cfg["pq_pool"] else nc.vector
            eng.tensor_mul(pq[:], h2[:], h13[:, 2 * C:4 * C])
        st["pq"] = pq
        return st

    def stage_b2(self, chs, sts):
        """Heads, ELU assembly, output DMA for a group of chunks sharing one
        psum head tile (amortizes ACT/DVE per-instruction overhead).

        Heads in a 37-row psum tile (weight columns pre-arranged host-side;
        rows 9:32 are zero columns):
          PH[0:9]   = [f1, f2, f2, f2, f2, wm1, wm2, wm3, wm4]
          PH[32:37] = [f1, f2, f3, f4, f5]   (g-rows for exp, 32-aligned)
        """
        nc, w, cfg = self.nc, self.w, self.cfg
        C = self.C
        zin, work, small, psum = self.pools
        g = len(chs)
        s0 = chs[0] * C

        ph = psum.tile([37, g * C], F32, tag="ph", bufs=cfg["ph_bufs"])
        for j, ch in enumerate(chs):
            pq = sts[j]["pq"]
            nc.tensor.matmul(ph[:, j * C:(j + 1) * C], w["wa37"][:],
                             pq[:, 0:C], start=True, stop=False)
            nc.tensor.matmul(ph[:, j * C:(j + 1) * C], w["wb37"][:],
                             pq[:, C:2 * C], start=False, stop=True)

        e = small.tile([5, g * C], F32, tag="e")
        nc.scalar.activation(e[:], ph[32:37, :], AF.Exp)
        t = small.tile([5, g * C], F32, tag="t")
        nc.vector.tensor_scalar(t[:], e[:], 1.0, 0.0, ALU.subtract, ALU.min)

        out9 = small.tile([9, g * C], F32, tag="out9")
        nc.vector.tensor_copy(out9[:], ph[0:9, :])
        nc.vector.scalar_tensor_tensor(out9[0:5, :], ph[0:5, :], 0.0,
                                       t[:], ALU.max, ALU.add)
        nc.sync.dma_start(self.dram["outT"][:, s0:s0 + g * C], out9[:])


def build_nc(npts=NPC, repeat=1, **overrides):
    cfg = dict(DEFAULT_CFG)
    cfg.update(overrides)
    c = cfg["c"]
    assert npts % c == 0
    nch = npts // c

    nc = bass.Bass(target_bir_lowering=False, trn_type="TRN2")
    dram = {
        "zt": nc.dram_tensor("zt", [15, npts], F32R, kind="ExternalInput"),
        "outT": nc.dram_tensor("outT", [9, npts], F32, kind="ExternalOutput"),
    }
    wshapes = {"wz": [15, 512], "w3a": [128, 256], "w3b": [128, 256],
               "wa37": [128, 37], "wb37": [128, 37]}
    wdram = {k: nc.dram_tensor(k, shp, F32R, kind="ExternalInput")
             for k, shp in wshapes.items()}

    with tile.TileContext(nc) as tc:
        with (
            tc.tile_pool(name="const", bufs=1) as const,
            tc.tile_pool(name="zin", bufs=cfg["zin_bufs"]) as zin,
            tc.tile_pool(name="work", bufs=cfg["work_bufs"]) as work,
            tc.tile_pool(name="small", bufs=cfg["small_bufs"]) as small,
            tc.tile_pool(name="psum", bufs=1, space="PSUM") as psum,
        ):
            w = {}
            for k, shp in wshapes.items():
                w[k] = const.tile(shp, F32R, tag=k, name=f"w_{k}")
                nc.sync.dma_start(w[k][:], wdram[k][:])

            stages = _Stages(nc, (zin, work, small, psum), w, dram, cfg)
            rep_ctx = (tc.For_i(0, repeat, 1) if repeat > 1
                       else contextlib.nullcontext())
            G = cfg["head_group"]
            assert nch % G == 0
            with rep_ctx:
                st = {}
                pend = []
                ahead = cfg["ahead"] if cfg["pipelined"] else 0
                for ch in range(nch + ahead):
                    if ch < nch:
                        st[ch] = stages.stage_a(ch)
                    cur = ch - ahead
                    if cur >= 0:
                        stages.stage_b1(cur, st[cur])
                        pend.append(cur)
                        if len(pend) == G:
                            stages.stage_b2(pend, [st.pop(p) for p in pend])
                            pend = []

    _split_multi_waits(nc)
    nc.finalize()
    return nc


_NC_CACHE = {}


def _get_nc(npts=NPC, repeat=1, **overrides):
    key = (npts, repeat, tuple(sorted(overrides.items())))
    if key not in _NC_CACHE:
        _NC_CACHE[key] = build_nc(npts, repeat=repeat, **overrides)
    return _NC_CACHE[key]


def _prep_weights(W1, W2, W3, Wp, Wq):
    wz = np.zeros((15, 512), dtype=np.float32)
    wz[0:9, 0:256] = W1
    wz[9:15, 256:512] = W2
    # op cols: f1,f2,f3,wm1,wm2 <- Wp ; oq cols: f4,f5,wm3,wm4 <- Wq
    # PH rows 0:9  = [f1, f2, f2, f2, f2, wm1, wm2, wm3, wm4]
    # PH rows 32:37 = [f1, f2, f3, f4, f5]
    wa37 = np.zeros((128, 37), dtype=np.float32)   # multiplies pp
    wb37 = np.zeros((128, 37), dtype=np.float32)   # multiplies qq
    wa37[:, 0] = Wp[:, 0]                     # f1
    for k in range(1, 5):
        wa37[:, k] = Wp[:, 1]                 # f2 x4
    wa37[:, 5] = Wp[:, 3]                     # wm1
    wa37[:, 6] = Wp[:, 4]                     # wm2
    wb37[:, 7] = Wq[:, 2]                     # wm3
    wb37[:, 8] = Wq[:, 3]                     # wm4
    wa37[:, 32:35] = Wp[:, 0:3]               # g: f1, f2, f3
    wb37[:, 35] = Wq[:, 0]                    # g: f4
    wb37[:, 36] = Wq[:, 1]                    # g: f5
    return {
        "wz": wz,
        "w3a": np.ascontiguousarray(W3[0:128, :], dtype=np.float32),
        "w3b": np.ascontiguousarray(W3[128:256, :], dtype=np.float32),
        "wa37": wa37, "wb37": wb37,
    }


def _run(x, y, W1, W2, W3, Wp, Wq, **spmd_kwargs):
    x = np.asarray(x, dtype=np.float32)
    y = np.asarray(y, dtype=np.float32)
    wmap = _prep_weights(np.asarray(W1, np.float32), np.asarray(W2, np.float32),
                         np.asarray(W3, np.float32), np.asarray(Wp, np.float32),
                         np.asarray(Wq, np.float32))

    nc = _get_nc()
    in_maps = []
    for c in range(N_CORES):
        sl = slice(c * NPC, (c + 1) * NPC)
        zt = np.empty((15, NPC), dtype=np.float32)
        zt[0:9] = x[sl].T
        zt[9:15] = y[sl].T
        m = {"zt": zt}
        m.update(wmap)
        in_maps.append(m)

    res = bass_utils.run_bass_kernel_spmd(nc, in_maps,
                                          core_ids=list(range(N_CORES)),
                                          **spmd_kwargs)
    out = np.empty((N_TOTAL, 9), dtype=np.float32)
    for c in range(N_CORES):
        out[c * NPC:(c + 1) * NPC] = res.results[c]["outT"].T
    return out, res


def kernel(x, y, W1, W2, W3, Wp, Wq):
    out, _ = _run(x, y, W1, W2, W3, Wp, Wq)
    return out
